# revision 9
# baseline (speedup 1.0000x reference)
"""Trainium2 Bass kernel: multi-head attention with RoPE.

Problem: y = softmax((RoPE(xWq^T) RoPE(xWk^T)^T)/sqrt(hd)) (xWv^T) Wo^T
  x [4, 2048, 1024], W* [1024, 1024], H=16 heads, hd=64.

Sharding: query-parallel, no collectives. Core c handles batch b=c//2 and
query rows [(c%2)*1024, (c%2+1)*1024) of that batch, against the batch's
full K/V (K/V projections are duplicated across the 2 cores per batch).
Each core returns a disjoint [1024, 1024] slice of the output; the host
concatenates.

On-chip layout is feature-major ("transposed") end to end, so no
transposes are ever needed:
  xT   [e, t]        Q'/K' [c, t] (head-dim on partitions)
  scoresT [k, q]     attnT [hd, t]  -> out-proj lhsT directly
Softmax runs over the partition axis: exp on ScalarE (scale=1/8 fused),
row-sums via ones-column matmuls on the PE, normalization deferred to
after the P@V matmul (divide the [hd, t] output by the broadcast
denominator; broadcast also via a tiny K=1 matmul).
"""

import numpy as np
import ml_dtypes

B, S, D = 4, 2048, 1024
H, HD = 16, 64
NCORES = 8
TQ = 1024
TK = 2048
NPAIR = 8
NKT = 16
BASE = 10000.0

BFNP = ml_dtypes.bfloat16

_STATE = None


def _emit(nc, tc, ctx):
    import concourse.mybir as mybir

    F32 = mybir.dt.float32
    BF16 = mybir.dt.bfloat16
    EXPF = mybir.ActivationFunctionType.Exp

    xT = nc.dram_tensor("xT", [D, TK], BF16, kind="ExternalInput").ap()
    wqT = nc.dram_tensor("wqT", [D, D], BF16, kind="ExternalInput").ap()
    wkT = nc.dram_tensor("wkT", [D, D], BF16, kind="ExternalInput").ap()
    wvT = nc.dram_tensor("wvT", [D, D], BF16, kind="ExternalInput").ap()
    woT = nc.dram_tensor("woT", [D, D], BF16, kind="ExternalInput").ap()
    cosK = nc.dram_tensor("cosK", [128, TK], F32, kind="ExternalInput").ap()
    sinK = nc.dram_tensor("sinK", [128, TK], F32, kind="ExternalInput").ap()
    rotT = nc.dram_tensor("rotT", [128, 128], BF16, kind="ExternalInput").ap()
    out = nc.dram_tensor("out", [TQ, D], F32, kind="ExternalOutput").ap()

    const = ctx.enter_context(tc.tile_pool(name="const", bufs=1))
    wq_pool = ctx.enter_context(tc.tile_pool(name="wq", bufs=2))
    wk_pool = ctx.enter_context(tc.tile_pool(name="wk", bufs=2))
    kq_pool = ctx.enter_context(tc.tile_pool(name="kq", bufs=2))
    rope_pool = ctx.enter_context(tc.tile_pool(name="rope", bufs=2))
    exp_pool = ctx.enter_context(tc.tile_pool(name="expp", bufs=4))
    norm_pool = ctx.enter_context(tc.tile_pool(name="norm", bufs=2))
    stage_pool = ctx.enter_context(tc.tile_pool(name="stage", bufs=2))

    # ---- persistent SBUF tensors ----
    xT_sb = const.tile([128, 8, TK], BF16, tag="xT")
    nc.sync.dma_start(xT_sb[:], xT.rearrange("(n p) t -> p n t", p=128))
    cosK_sb = const.tile([128, TK], F32, tag="cosK")
    sinK_sb = const.tile([128, TK], F32, tag="sinK")
    nc.sync.dma_start(cosK_sb[:], cosK[:])
    nc.sync.dma_start(sinK_sb[:], sinK[:])
    rotT_sb = const.tile([128, 128], BF16, tag="rotT")
    nc.sync.dma_start(rotT_sb[:], rotT[:])
    wv_sb = const.tile([128, 8, D], BF16, tag="wv")
    nc.sync.dma_start(wv_sb[:], wvT.rearrange("(n p) c -> p n c", p=128))
    ones_sb = const.tile([128, 1], BF16, tag="ones")
    nc.gpsimd.memset(ones_sb[:], 1.0)
    ones64_sb = const.tile([128, 64], BF16, tag="ones64")
    nc.gpsimd.memset(ones64_sb[:], 1.0)
    v_sb = const.tile([128, NKT, 16 * 65], BF16, tag="v")
    v_aug = v_sb[:].rearrange("p t (h c) -> p t h c", c=65)
    nc.gpsimd.memset(v_aug[:, :, :, 64:65], 1.0)
    attnT_sb = const.tile([128, NPAIR, TQ], BF16, tag="attnT")

    with (
        tc.tile_pool(name="psum_proj", bufs=2, space="PSUM") as proj_psum,
        tc.tile_pool(name="psum_rot", bufs=1, space="PSUM") as rot_psum,
        tc.tile_pool(name="psum_sc", bufs=2, space="PSUM") as score_psum,
        tc.tile_pool(name="psum_pv", bufs=1, space="PSUM") as pv_psum,
        tc.tile_pool(name="psum_bc", bufs=1, space="PSUM") as bc_psum,
    ):

        def emit_v_tile(tk):
            for cc in range(2):
                ps = proj_psum.tile([128, 512], F32, tag="proj", name=f"vps_{tk}_{cc}")
                for e in range(8):
                    nc.tensor.matmul(
                        ps[:],
                        xT_sb[:, e, tk * 128 : (tk + 1) * 128],
                        wv_sb[:, e, cc * 512 : (cc + 1) * 512],
                        start=(e == 0),
                        stop=(e == 7),
                    )
                nc.scalar.copy(v_aug[:, tk, cc * 8 : (cc + 1) * 8, 0:64], ps[:])

        def emit_proj_pair(p):
            """K'/Q' (RoPE'd, feature-major, bf16) for head pair p."""
            wkt = wk_pool.tile([128, 8, 128], BF16, tag="wk")
            nc.sync.dma_start(
                wkt[:],
                wkT[:, p * 128 : (p + 1) * 128].rearrange("(n p) c -> p n c", p=128),
            )
            wqt = wq_pool.tile([128, 8, 128], BF16, tag="wq")
            nc.sync.dma_start(
                wqt[:],
                wqT[:, p * 128 : (p + 1) * 128].rearrange("(n p) c -> p n c", p=128),
            )
            ktp = kq_pool.tile([128, TK], BF16, tag="ktp")
            qtp = kq_pool.tile([128, TQ], BF16, tag="qtp")
            for w_sb, dst, nt, cos_sb, sin_sb in (
                (wkt, ktp, 4, cosK_sb, sinK_sb),
                (wqt, qtp, 2, cosK_sb, sinK_sb),
            ):
                for tt in range(nt):
                    sl = slice(tt * 512, (tt + 1) * 512)
                    ps = proj_psum.tile([128, 512], F32, tag="proj", name=f"pps_{p}_{tt}_{nt}")
                    for e in range(8):
                        nc.tensor.matmul(
                            ps[:],
                            w_sb[:, e, :],
                            xT_sb[:, e, sl],
                            start=(e == 0),
                            stop=(e == 7),
                        )
                    a = rope_pool.tile([128, 512], F32, tag="ra")
                    nc.vector.tensor_mul(a[:], ps[:], cos_sb[:, sl])
                    s = rope_pool.tile([128, 512], BF16, tag="rs")
                    nc.vector.tensor_mul(s[:], ps[:], sin_sb[:, sl])
                    rp = rot_psum.tile([128, 512], F32, tag="rot")
                    nc.tensor.matmul(rp[:], rotT_sb[:], s[:], start=True, stop=True)
                    nc.vector.tensor_add(dst[:, sl], a[:], rp[:])
            return ktp, qtp

        def emit_attn_pair(p, ktp, qtp, with_v):
            """Scores/exp/PV (V-augmented: row 64 of pv accumulates the softmax
            denominator). Evacuates unnormalized pv and denominator rows;
            normalization is emitted later (emit_norm) to keep PE busy."""
            os_ = []
            dens_list = []
            rdens_list = []
            for tq in range(2):
                qsl = slice(tq * 512, (tq + 1) * 512)
                pva = pv_psum.tile([128, 512], F32, tag="pva")
                pvb = pv_psum.tile([128, 512], F32, tag="pvb")
                for tk in range(NKT):
                    if with_v and tq == 0:
                        emit_v_tile(tk)
                    ksl = slice(tk * 128, (tk + 1) * 128)
                    first = tk == 0
                    last = tk == NKT - 1
                    sa = score_psum.tile([128, 512], F32, tag="sc")
                    nc.tensor.matmul(
                        sa[:], ktp[0:64, ksl], qtp[0:64, qsl], start=True, stop=True
                    )
                    sb_ = score_psum.tile([128, 512], F32, tag="sc")
                    nc.tensor.matmul(
                        sb_[:], ktp[64:128, ksl], qtp[64:128, qsl], start=True, stop=True
                    )
                    ea = exp_pool.tile([128, 512], BF16, tag="exp")
                    nc.scalar.activation(ea[:], sa[:], EXPF, scale=0.125)
                    eb = exp_pool.tile([128, 512], BF16, tag="exp")
                    nc.scalar.activation(eb[:], sb_[:], EXPF, scale=0.125)
                    nc.tensor.matmul(
                        pva[0:65, :],
                        v_aug[:, tk, 2 * p, :],
                        ea[:],
                        start=first,
                        stop=last,
                    )
                    nc.tensor.matmul(
                        pvb[0:65, :],
                        v_aug[:, tk, 2 * p + 1, :],
                        eb[:],
                        start=first,
                        stop=last,
                    )
                o = norm_pool.tile([128, 512], F32, tag="o", bufs=3)
                nc.vector.tensor_copy(o[0:64, :], pva[0:64, :])
                nc.vector.tensor_copy(o[64:128, :], pvb[0:64, :])
                dens = norm_pool.tile([128, 512], F32, tag="dens", bufs=3)
                nc.gpsimd.memset(dens[:], 1.0)
                nc.vector.tensor_copy(dens[0:1, :], pva[64:65, :])
                nc.vector.tensor_copy(dens[32:33, :], pvb[64:65, :])
                os_.append(o)
                dens_list.append(dens)
            for dens in dens_list:
                rdens = norm_pool.tile([128, 512], BF16, tag="rdens", bufs=3)
                with nc.allow_low_precision(reason="bf16 softmax denominators"):
                    nc.vector.reciprocal(rdens[0:33, :], dens[0:33, :])
                rdens_list.append(rdens)
            return os_, rdens_list

        def emit_norm(p, os_, rdens_list):
            for tq in range(2):
                qsl = slice(tq * 512, (tq + 1) * 512)
                rdens = rdens_list[tq]
                bc = bc_psum.tile([128, 512], F32, tag="bc")
                nc.tensor.matmul(
                    bc[0:64, :],
                    ones64_sb[0:1, :],
                    rdens[0:1, :],
                    start=True,
                    stop=True,
                )
                nc.tensor.matmul(
                    bc[64:128, :],
                    ones64_sb[32:33, :],
                    rdens[32:33, :],
                    start=True,
                    stop=True,
                    skip_group_check=True,
                )
                nc.vector.tensor_mul(attnT_sb[:, p, qsl], os_[tq][:], bc[:])

        pending = None
        for p in range(NPAIR):
            ktp, qtp = emit_proj_pair(p)
            if pending is not None:
                emit_norm(*pending)
            os_, rdens = emit_attn_pair(p, ktp, qtp, with_v=(p == 0))
            pending = (p, os_, rdens)
        emit_norm(*pending)

    # ---- output projection: out[t, e] = sum_hd attnT[hd, t] * woT[hd, e] ----
    with (
        tc.tile_pool(name="psum_out", bufs=8, space="PSUM") as outp_psum,
        tc.tile_pool(name="wo", bufs=2) as wo_pool,
    ):
        for half in range(2):
            pss = [
                outp_psum.tile([128, 512], F32, tag="po", name=f"po_{half}_{i}")
                for i in range(8)
            ]
            for hp in range(8):
                wot = wo_pool.tile([128, D], BF16, tag="wo")
                nc.sync.dma_start(wot[:], woT[hp * 128 : (hp + 1) * 128, :])
                for ti in range(4):
                    tt = half * 4 + ti
                    tsl = slice(tt * 128, (tt + 1) * 128)
                    for ec in range(2):
                        nc.tensor.matmul(
                            pss[ti * 2 + ec][:],
                            attnT_sb[:, hp, tsl],
                            wot[:, ec * 512 : (ec + 1) * 512],
                            start=(hp == 0),
                            stop=(hp == 7),
                        )
            for ti in range(4):
                tt = half * 4 + ti
                st = stage_pool.tile([128, D], F32, tag="st")
                nc.scalar.copy(st[:, 0:512], pss[ti * 2][:])
                nc.scalar.copy(st[:, 512:1024], pss[ti * 2 + 1][:])
                nc.sync.dma_start(out[tt * 128 : (tt + 1) * 128, :], st[:])


def _build_nc():
    from contextlib import ExitStack

    import concourse.bacc as bacc
    import concourse.tile as tile

    nc = bacc.Bacc("TRN2", target_bir_lowering=False, debug=False)
    with tile.TileContext(nc) as tc:
        with ExitStack() as ctx:
            _emit(nc, tc, ctx)
    nc.compile()
    return nc


def _rope_tables():
    inv = 1.0 / (BASE ** (np.arange(0, HD, 2, dtype=np.float64) / HD))  # [32]
    t = np.arange(S, dtype=np.float64)
    ang = np.outer(t, inv)  # [S, 32]
    cos64 = np.concatenate([np.cos(ang), np.cos(ang)], axis=1).T  # [64, S]
    sin64 = np.concatenate([np.sin(ang), np.sin(ang)], axis=1).T
    cos128 = np.tile(cos64, (2, 1)).astype(np.float32)  # [128, S]
    sin128 = np.tile(sin64, (2, 1)).astype(np.float32)
    return cos128, sin128


def _rot_matrix():
    R = np.zeros((128, 128), dtype=np.float32)
    for blk in (0, 64):
        for d in range(32):
            R[blk + d, blk + d + 32] = -1.0
            R[blk + d + 32, blk + d] = 1.0
    return np.ascontiguousarray(R.T).astype(BFNP)


def _make_runner(nc):
    import jax
    from jax.sharding import Mesh, PartitionSpec
    from jax.experimental.shard_map import shard_map

    from concourse import bass2jax, mybir

    bass2jax.install_neuronx_cc_hook()
    partition_name = (
        nc.partition_id_tensor.name if nc.partition_id_tensor else None
    )
    in_names, out_names, out_avals, zero_outs = [], [], [], []
    for alloc in nc.m.functions[0].allocations:
        if not isinstance(alloc, mybir.MemoryLocationSet):
            continue
        name = alloc.memorylocations[0].name
        if alloc.kind == "ExternalInput":
            if name != partition_name:
                in_names.append(name)
        elif alloc.kind == "ExternalOutput":
            shape = tuple(alloc.tensor_shape)
            dtype = mybir.dt.np(alloc.dtype)
            out_names.append(name)
            out_avals.append(jax.core.ShapedArray(shape, dtype))
            zero_outs.append(np.zeros(shape, dtype))
    n_params = len(in_names)
    n_outs = len(out_avals)
    all_in_names = list(in_names) + list(out_names)
    if partition_name is not None:
        all_in_names.append(partition_name)
    donate = tuple(range(n_params, n_params + n_outs))

    def _body(*args):
        operands = list(args)
        if partition_name is not None:
            operands.append(bass2jax.partition_id_tensor())
        outs = bass2jax._bass_exec_p.bind(
            *operands,
            out_avals=tuple(out_avals),
            in_names=tuple(all_in_names),
            out_names=tuple(out_names),
            lowering_input_output_aliases=(),
            sim_require_finite=True,
            sim_require_nnan=True,
            nc=nc,
        )
        return tuple(outs)

    devices = jax.devices()[:NCORES]
    mesh = Mesh(np.asarray(devices), ("core",))
    in_specs = (PartitionSpec("core"),) * (n_params + n_outs)
    out_specs = (PartitionSpec("core"),) * n_outs
    sharded = jax.jit(
        shard_map(
            _body, mesh=mesh, in_specs=in_specs, out_specs=out_specs, check_rep=False
        ),
        donate_argnums=donate,
        keep_unused=True,
    )

    def run(in_maps):
        per_core = [[np.asarray(m[name]) for name in in_names] for m in in_maps]
        concat_in = [
            np.concatenate([per_core[c][i] for c in range(NCORES)], axis=0)
            for i in range(n_params)
        ]
        concat_zeros = [
            np.zeros((NCORES * z.shape[0], *z.shape[1:]), z.dtype) for z in zero_outs
        ]
        out_arrs = sharded(*concat_in, *concat_zeros)
        return [
            {
                name: np.asarray(out_arrs[i]).reshape(
                    NCORES, *out_avals[i].shape
                )[c]
                for i, name in enumerate(out_names)
            }
            for c in range(NCORES)
        ]

    return run


def _get_state():
    global _STATE
    if _STATE is None:
        nc = _build_nc()
        run = _make_runner(nc)
        _STATE = (nc, run)
    return _STATE


def _in_maps(x, Wq, Wk, Wv, Wo):
    cos128, sin128 = _rope_tables()
    rotT = _rot_matrix()
    wqT = np.ascontiguousarray(Wq.T).astype(BFNP)
    wkT = np.ascontiguousarray(Wk.T).astype(BFNP)
    wvT = np.ascontiguousarray(Wv.T).astype(BFNP)
    woT = np.ascontiguousarray(Wo.T).astype(BFNP)
    xT = np.ascontiguousarray(np.asarray(x).transpose(0, 2, 1)).astype(BFNP)
    maps = []
    for c in range(NCORES):
        b, qh = c // 2, c % 2
        qoff = qh * TQ
        # Rotate the key/value token axis so this core's query block is
        # always columns [0, TQ). Attention is permutation-invariant in k.
        maps.append(
            {
                "xT": np.ascontiguousarray(np.roll(xT[b], -qoff, axis=1)),
                "wqT": wqT,
                "wkT": wkT,
                "wvT": wvT,
                "woT": woT,
                "cosK": np.ascontiguousarray(np.roll(cos128, -qoff, axis=1)),
                "sinK": np.ascontiguousarray(np.roll(sin128, -qoff, axis=1)),
                "rotT": rotT,
            }
        )
    return maps


def kernel(x, Wq, Wk, Wv, Wo):
    _, run = _get_state()
    results = run(_in_maps(x, Wq, Wk, Wv, Wo))
    y = np.empty((B, S, D), dtype=np.float32)
    for c in range(NCORES):
        b, qh = c // 2, c % 2
        y[b, qh * TQ : (qh + 1) * TQ, :] = results[c]["out"]
    return y
